# revision 6
# baseline (speedup 1.0000x reference)
"""KeyFormer sparse-attention kernel for Trainium2 (Bass/Tile), 8-core SPMD.

Reference computation (per batch=1):
  scores = q @ k^T / sqrt(D)            [H, QL, KL], causal-masked
  probs  = softmax(scores)              -> out = probs @ v        [QL, H, D]
  kf     = softmax((scores - ln er)/tau).sum(axis=q)              [H, KL]

Sharding: 32 heads across 8 cores (4 heads/core), zero communication.

Per-core kernel layout (per head, fully unrolled):
  - q/k blocks loaded naturally, PE-transposed to qT/kT [D, QL] strips.
  - S accumulated per q-block causal strip [128, L] in PSUM via f32r matmuls;
    the causal diagonal mask is added with a bf16 identity-matmul.
  - E = exp(S/sqrt(D)) on ACT (PSUM->SBUF) with fused accum row-sum -> r.
  - Gumbel term added in PSUM via (-sqrt(D)*I) @ ln(er) matmul, then
    G = exp(S~/(sqrt(D)*tau)) with fused accum -> s.
  - kf accumulated in PSUM: (1/s)^T @ G matmuls over q-blocks.
  - out^T accumulated from PE-transposed E strips @ v, normalized by 1/r
    after a final PE transpose back to [q, d].
"""

import math
import os
import numpy as np
from contextlib import ExitStack

import concourse.bacc as bacc
import concourse.tile as tile
import concourse.bass as bass
from concourse import mybir, bass_utils

F32 = mybir.dt.float32
F32R = mybir.dt.float32r
BF16 = mybir.dt.bfloat16

B, QL, KL, H, D = 1, 1024, 1024, 32, 128
TAU = 1.5
NCORES = 8
HPC = H // NCORES  # heads per core
NQB = QL // 128    # q blocks per head
SQD = math.sqrt(D)
MASK_NEG = -1.0e30

# causal strip length and packed offset per q-block
L_OF = [128 * (qi + 1) for qi in range(NQB)]
OFF_OF = [sum(L_OF[:qi]) for qi in range(NQB)]
LTOT = sum(L_OF)  # 4608


def _chunks(lo, hi, step=512):
    out = []
    c = lo
    while c < hi:
        out.append((c, min(step, hi - c)))
        c += step
    return out


def build_kernel(use_f32r=True):
    MMDT = F32R if use_f32r else F32
    nc = bacc.Bacc("TRN2", target_bir_lowering=False, debug=False)

    q = nc.dram_tensor("q", [QL, HPC, D], F32, kind="ExternalInput")
    k = nc.dram_tensor("k", [KL, HPC, D], F32, kind="ExternalInput")
    v = nc.dram_tensor("v", [KL, HPC, D], F32, kind="ExternalInput")
    er = nc.dram_tensor("er", [HPC, QL, KL], F32, kind="ExternalInput")
    # consts_f32: [ -sqrt(D)*I | I | zeros-col ]  -> [128, 257]
    cf = nc.dram_tensor("cf", [128, 257], F32, kind="ExternalInput")
    # consts_bf16: [ I | diag-causal-mask ] -> [128, 256]
    cb = nc.dram_tensor("cb", [128, 256], BF16, kind="ExternalInput")
    out = nc.dram_tensor("out", [QL, HPC, D], F32, kind="ExternalOutput")
    kf = nc.dram_tensor("kf", [HPC, KL], F32, kind="ExternalOutput")

    with tile.TileContext(nc) as tc, ExitStack() as ctx:
        consts = ctx.enter_context(tc.tile_pool(name="consts", bufs=1))
        qknat = ctx.enter_context(tc.tile_pool(name="qknat", bufs=4))
        qkt = ctx.enter_context(tc.tile_pool(name="qkt", bufs=2))
        vpool = ctx.enter_context(tc.tile_pool(name="vpool", bufs=2))
        erpool = ctx.enter_context(tc.tile_pool(name="erpool", bufs=2))
        lnrpool = ctx.enter_context(tc.tile_pool(name="lnrpool", bufs=2))
        egpool = ctx.enter_context(tc.tile_pool(name="egpool", bufs=3))
        etpool = ctx.enter_context(tc.tile_pool(name="etpool", bufs=2))
        smalls = ctx.enter_context(tc.tile_pool(name="smalls", bufs=8))
        outsb = ctx.enter_context(tc.tile_pool(name="outsb", bufs=2))
        ops = ctx.enter_context(tc.tile_pool(name="ops", bufs=3))

        spool = ctx.enter_context(tc.tile_pool(name="spool", bufs=2, space="PSUM"))
        tpool = ctx.enter_context(tc.tile_pool(name="tpool", bufs=2, space="PSUM"))
        kfpool = ctx.enter_context(tc.tile_pool(name="kfpool", bufs=1, space="PSUM"))

        # --- constants ---
        negsqdI = consts.tile([128, 128], MMDT, tag="negsqdI")
        nc.sync.dma_start(negsqdI[:], cf.ap()[:, 0:128].bitcast(MMDT))
        identm = consts.tile([128, 128], MMDT, tag="identm")
        nc.sync.dma_start(identm[:], cf.ap()[:, 128:256].bitcast(MMDT))
        zcol = consts.tile([128, 1], MMDT, tag="zcol")
        nc.sync.dma_start(zcol[:], cf.ap()[:, 256:257].bitcast(MMDT))
        identf32 = consts.tile([128, 128], F32, tag="identf32")
        nc.sync.dma_start(identf32[:], cf.ap()[:, 128:256])
        identb = consts.tile([128, 128], BF16, tag="identb")
        nc.sync.dma_start(identb[:], cb.ap()[:, 0:128])
        maskb = consts.tile([128, 128], BF16, tag="maskb")
        nc.sync.dma_start(maskb[:], cb.ap()[:, 128:256])

        for h in range(HPC):
            # --- transposed loads: qT/kT [D, QL] strips ---
            qT = qkt.tile([128, QL], MMDT, tag="qT")
            kT = qkt.tile([128, QL], MMDT, tag="kT")
            for b in range(NQB):
                for name, src, dst in (("q", q, qT), ("k", k, kT)):
                    nat = qknat.tile([128, 128], MMDT, tag="nat")
                    nc.sync.dma_start(
                        nat[:], src.ap()[128 * b : 128 * (b + 1), h, :].bitcast(MMDT)
                    )
                    tps = tpool.tile([128, 128], MMDT, tag="tps")
                    nc.tensor.transpose(tps[:], nat[:], identm[:])
                    nc.vector.tensor_copy(dst[:, 128 * b : 128 * (b + 1)], tps[:])

            # v natural [k, d] blocks packed [128, 8*128] (kb-major)
            vsb = vpool.tile([128, NQB * 128], MMDT, tag="vsb")
            for kb in range(NQB):
                nc.sync.dma_start(
                    vsb[:, 128 * kb : 128 * (kb + 1)],
                    v.ap()[128 * kb : 128 * (kb + 1), h, :].bitcast(MMDT),
                )

            # er packed causal [128, LTOT]; ln(er) in two halves
            ersb = erpool.tile([128, LTOT], F32, tag="ersb")
            for qi in range(NQB):
                nc.sync.dma_start(
                    ersb[:, OFF_OF[qi] : OFF_OF[qi] + L_OF[qi]],
                    er.ap()[h, 128 * qi : 128 * (qi + 1), 0 : L_OF[qi]],
                )
            lnr = lnrpool.tile([128, LTOT], MMDT, tag="lnr")
            half = LTOT // 2
            nc.scalar.activation(
                lnr[:, 0:half], ersb[:, 0:half], mybir.ActivationFunctionType.Ln
            )
            nc.scalar.activation(
                lnr[:, half:LTOT], ersb[:, half:LTOT], mybir.ActivationFunctionType.Ln
            )

            # kf accumulator [1, 1024] psum (2 banks, partition 0)
            kft = kfpool.tile([1, 1024], F32, tag="kft")
            nc.tensor.matmul(
                kft[0:1, 0:512], zcol[:], kT[:, 0:512], start=True, stop=False,
                skip_group_check=True,
            )
            nc.tensor.matmul(
                kft[0:1, 512:1024], zcol[:], kT[:, 512:1024], start=True, stop=False,
                skip_group_check=True,
            )

            # E^T strips per k-block: [128, (8-kb)*128]
            ets = [
                etpool.tile(
                    [128, (NQB - kb) * 128], MMDT, tag=f"et{kb}", name=f"et{kb}"
                )
                for kb in range(NQB)
            ]
            rrec = []  # 1/r per q-block

            for qi in range(NQB):
                L = L_OF[qi]
                off = OFF_OF[qi]
                sps = spool.tile([128, 1024], F32, tag="sps")
                # S = qT_qi^T @ kT (causal chunks) + diag mask
                for c0, cl in _chunks(0, L):
                    last = c0 + cl == L
                    nc.tensor.matmul(
                        sps[:, c0 : c0 + cl],
                        qT[:, 128 * qi : 128 * (qi + 1)],
                        kT[:, c0 : c0 + cl],
                        start=True,
                        stop=not last,
                        skip_group_check=True,
                    )
                nc.tensor.matmul(
                    sps[:, L - 128 : L],
                    identb[:],
                    maskb[:],
                    start=False,
                    stop=True,
                    skip_group_check=True,
                )
                # E = exp(S/sqrt(D)), accum -> r
                esb = egpool.tile([128, 1024], MMDT, tag="esb")
                racc = smalls.tile([128, 1], F32, tag="racc")
                nc.scalar.activation(
                    esb[:, 0:L],
                    sps[:, 0:L],
                    mybir.ActivationFunctionType.Exp,
                    scale=1.0 / SQD,
                    accum_out=racc[:],
                )
                rr = smalls.tile([128, 1], F32, tag="rr")
                nc.vector.reciprocal(rr[:], racc[:])
                rrec.append(rr)
                # S~ = S - sqrt(D)*ln(er)
                for c0, cl in _chunks(0, L):
                    nc.tensor.matmul(
                        sps[:, c0 : c0 + cl],
                        negsqdI[:],
                        lnr[:, off + c0 : off + c0 + cl],
                        start=False,
                        stop=True,
                        skip_group_check=True,
                    )
                # G = exp(S~/(sqrt(D)*tau)), accum -> s
                gsb = egpool.tile([128, 1024], MMDT, tag="gsb")
                sacc = smalls.tile([128, 1], F32, tag="sacc")
                nc.scalar.activation(
                    gsb[:, 0:L],
                    sps[:, 0:L],
                    mybir.ActivationFunctionType.Exp,
                    scale=1.0 / (SQD * TAU),
                    accum_out=sacc[:],
                )
                srec = smalls.tile([128, 1], MMDT, tag="srec")
                with nc.allow_low_precision(reason="1/s feeds f32r matmul"):
                    nc.vector.reciprocal(srec[:], sacc[:])
                # kf += (1/s)^T @ G
                for c0, cl in _chunks(0, L):
                    tgt = kft[0:1, c0 : c0 + cl]
                    nc.tensor.matmul(
                        tgt,
                        srec[:],
                        gsb[:, c0 : c0 + cl],
                        start=False,
                        stop=(qi == NQB - 1) and (c0 + cl == L),
                        skip_group_check=True,
                    )
                # E^T blocks for PV
                for kb in range(qi + 1):
                    tps = tpool.tile([128, 128], MMDT, tag="tps")
                    nc.tensor.transpose(
                        tps[:], esb[:, 128 * kb : 128 * (kb + 1)], identm[:]
                    )
                    nc.vector.tensor_copy(
                        ets[kb][:, 128 * (qi - kb) : 128 * (qi - kb + 1)], tps[:]
                    )

            # PV: out^T [d, q] per 512-q half, then transpose+normalize+store
            for qc in range(2):
                otp = spool.tile([128, 1024], F32, tag="sps", name="otp")
                kbmax = 4 * qc + 3
                for kb in range(kbmax + 1):
                    qlo = max(512 * qc, 128 * kb)
                    qhi = 512 * (qc + 1)
                    loc = qlo - 128 * kb
                    nc.tensor.matmul(
                        otp[:128, qlo - 512 * qc : qhi - 512 * qc],
                        vsb[:, 128 * kb : 128 * (kb + 1)],
                        ets[kb][:, loc : loc + (qhi - qlo)],
                        start=(kb == 0),
                        stop=(kb == kbmax),
                        skip_group_check=True,
                    )
                otsb = outsb.tile([128, 512], F32, tag="otsb")
                nc.vector.tensor_copy(otsb[:], otp[:, 0:512])
                for j in range(4):
                    qb = 4 * qc + j
                    ops_t = tpool.tile([128, 128], F32, tag="tps")
                    nc.tensor.transpose(
                        ops_t[:], otsb[:, 128 * j : 128 * (j + 1)], identf32[:]
                    )
                    osb = ops.tile([128, 128], F32, tag="osb")
                    nc.vector.tensor_scalar_mul(osb[:], ops_t[:], rrec[qb][:])
                    nc.sync.dma_start(
                        out.ap()[128 * qb : 128 * (qb + 1), h, :], osb[:]
                    )

            # kf out (PSUM -> SBUF -> DRAM)
            kfs = smalls.tile([1, 1024], F32, tag="kfs")
            nc.vector.tensor_copy(kfs[:], kft[:])
            nc.sync.dma_start(kf.ap()[h, :], kfs[0:1, :])

    nc.compile()
    return nc


def _consts_np():
    ident = np.eye(128, dtype=np.float32)
    cf = np.concatenate(
        [-SQD * ident, ident, np.zeros((128, 1), np.float32)], axis=1
    )
    import ml_dtypes

    mask = np.where(
        np.arange(128)[:, None] >= np.arange(128)[None, :], 0.0, MASK_NEG
    ).astype(np.float32)
    cb = np.concatenate([ident, mask], axis=1).astype(ml_dtypes.bfloat16)
    return cf, cb


_CACHE = {}


def _get_nc(use_f32r=True):
    key = bool(use_f32r)
    if key not in _CACHE:
        _CACHE[key] = build_kernel(use_f32r=key)
    return _CACHE[key]


def kernel(q, k, v, exp_rand, _trace=False, _use_f32r=None):
    if _use_f32r is None:
        _use_f32r = os.environ.get("KF_F32R", "1") == "1"
    nc = _get_nc(_use_f32r)
    cf, cb = _consts_np()
    q = np.asarray(q, dtype=np.float32)
    k = np.asarray(k, dtype=np.float32)
    v = np.asarray(v, dtype=np.float32)
    exp_rand = np.asarray(exp_rand, dtype=np.float32)

    in_maps = []
    for c in range(NCORES):
        hs = slice(HPC * c, HPC * (c + 1))
        in_maps.append(
            {
                "q": np.ascontiguousarray(q[0, :, hs, :]),
                "k": np.ascontiguousarray(k[0, :, hs, :]),
                "v": np.ascontiguousarray(v[0, :, hs, :]),
                "er": np.ascontiguousarray(exp_rand[0, hs, :, :]),
                "cf": cf,
                "cb": cb,
            }
        )
    res = bass_utils.run_bass_kernel_spmd(
        nc, in_maps, core_ids=list(range(NCORES)), trace=_trace
    )
    out = np.empty((B, QL, H, D), np.float32)
    kf = np.empty((B, H, KL), np.float32)
    for c in range(NCORES):
        hs = slice(HPC * c, HPC * (c + 1))
        out[0, :, hs, :] = res.results[c]["out"]
        kf[0, hs, :] = res.results[c]["kf"]
    if _trace:
        return (out, kf), res
    return out, kf
